# revision 27
# baseline (speedup 1.0000x reference)
"""Haar DWT2D (pywt even-size convention) on 8 Trainium2 NeuronCores.

Input  x: (16, 64, 512, 512) f32.
Output (LL, (LH, HL, HH)), each (16, 64, 256, 256) f32.

Sharding: pure data parallel over batch — core i handles x[2i:2i+2].

Per-core layout: the (2, 64, 512, 512) slice is viewed flat as N_TILES tiles
of [128 partitions x TILE_FREE f32]. One partition line = TILE_FREE/512
consecutive image rows (even/odd row pairs; the row count divides 512 so a
line never straddles images and always starts on an even row). Two-stage
butterfly per tile:
  ACT  :  Oh = 0.5*O                         (scalar engine, frees DVE)
  DVE 1:  S = 0.5*E + Oh;  D = 0.5*E - Oh    (row direction; D is written
          into X's odd-row region — D' reads only E and Oh, so the slot is
          reused in place and one SBUF buffer is saved)
  DVE 2:  LL = Se + So, HL = Se - So, LH = De + Do, HH = De - Do
          (column direction, stride-2 reads / contiguous writes)
The four outputs are packed side by side in one [128, TILE_FREE] SBUF tile
and leave in a single DMA per tile to a quadrant-major y DRAM tensor: the
resulting 8 KB write descriptors interleave measurably better with the
32 KB read descriptors than one contiguous 32 KB line per partition would
(A/B measured: 684 us vs 835 us).

Raw bass (no Tile): walrus' CoreV2/V3 descriptors have a single embedded
sync-wait slot, so all waits are standalone wait_ge instructions on the
issuing engine. SP sequencer does input DMAs, ACT does output DMAs + Oh,
DVE does the butterflies. X is triple-buffered, OUT double-buffered.

Sync rules learned from CoreSim's race detector + walrus:
- compute engines have no RAW/WAR interlock between their own pipelined
  instructions: a read (or overwrite) of a buffer touched by an earlier
  instruction on the SAME engine needs a standalone wait on that
  instruction's retire increment (engines retire in order, so one sem
  per chain checkpoint suffices);
- a counting DMA semaphore may only track one in-flight DMA (16 engine
  increments are indistinguishable between two half-done transfers), so
  each buffer slot gets its own DMA semaphore;
- an engine must have observed a semaphore's current value before
  re-incrementing it (hence the sem_in/sem_out "observe" waits).
"""

from contextlib import ExitStack

import numpy as np

from concourse import bass, mybir
from concourse.bass_utils import run_bass_kernel_spmd

N_CORES = 8
B, C, H, W = 16, 64, 512, 512
PER_CORE_B = B // N_CORES  # 2
TILE_FREE = 8192  # f32 per partition line (32 KB) -> 4 MiB per DMA
N_TILES = PER_CORE_B * C * H * W // (128 * TILE_FREE)  # 32
OUT_NAMES = ("ll", "lh", "hl", "hh")

FP32 = mybir.dt.float32
NBX = 3  # X (input) buffer depth
NBO = 2  # OUT buffer depth


def _build_kernel(n_tiles: int = N_TILES) -> bass.Bass:
    nc = bass.Bass()
    x = nc.dram_tensor("x", [n_tiles, 128, TILE_FREE], FP32, kind="ExternalInput")
    y = nc.dram_tensor(
        "y", [4, n_tiles, 128, TILE_FREE // 4], FP32, kind="ExternalOutput"
    )
    # DMA view: for tile t, [partition, quadrant, column]
    y_view = y[:].rearrange("q n p m -> n p q m")

    with ExitStack() as ctx:
        X = [
            ctx.enter_context(nc.sbuf_tensor(f"xb{i}", [128, TILE_FREE], FP32))
            for i in range(NBX)
        ]
        OUT = [
            ctx.enter_context(nc.sbuf_tensor(f"ob{i}", [128, TILE_FREE], FP32))
            for i in range(NBO)
        ]
        S = ctx.enter_context(nc.sbuf_tensor("sbuf_s", [128, TILE_FREE // 2], FP32))
        OH = ctx.enter_context(nc.sbuf_tensor("sbuf_oh", [128, TILE_FREE // 2], FP32))
        sem_in = [
            ctx.enter_context(nc.semaphore(f"sem_in{i}")) for i in range(NBX)
        ]
        sem_out = [
            ctx.enter_context(nc.semaphore(f"sem_out{i}")) for i in range(NBO)
        ]
        # sem_v: D' retired (stage 2 may read S and D-in-X; Oh is free)
        # sem_act: Oh written by ACT, read by DVE's S'/D'
        # sem_dve: stage 2 retired (out-DMA may read OUT; X slot is free)
        sem_v = ctx.enter_context(nc.semaphore("sem_v"))
        sem_act = ctx.enter_context(nc.semaphore("sem_act"))
        sem_dve = ctx.enter_context(nc.semaphore("sem_dve"))
        block = ctx.enter_context(nc.Block())

        def out_dma(scalar, t):
            i = t % NBO
            scalar.wait_ge(sem_dve, t + 1)
            if t >= NBO:
                scalar.wait_ge(sem_out[i], 16 * (t // NBO))
            ob = OUT[i][:].rearrange("p (q m) -> p q m", m=TILE_FREE // 4)
            scalar.dma_start(out=y_view[t], in_=ob).then_inc(sem_out[i], 16)

        @block.sync
        def _(sync):
            for t in range(n_tiles):
                i = t % NBX
                if t >= NBX:
                    # X[i] holds D of iteration t-NBX until stage 2 reads
                    # it, and ACT reads its odd rows for Oh
                    sync.wait_ge(sem_dve, t - NBX + 1)
                    sync.wait_ge(sem_act, t - NBX + 1)
                    # observe this slot's previous DMA completion (implied
                    # by the waits above, but the sem update rule wants the
                    # issuing engine to have seen the current value)
                    sync.wait_ge(sem_in[i], 16 * (t // NBX))
                sync.dma_start(out=X[i][:], in_=x[t]).then_inc(sem_in[i], 16)

        @block.scalar
        def _(scalar):
            # ACT: compute Oh(t) = 0.5 * O(t) early each iteration, then
            # issue the out-DMA for tile t-1. Oh(t) only needs X(t) and the
            # retire of D'(t-1), so it never waits behind DVE's stage 2.
            for t in range(n_tiles):
                scalar.wait_ge(sem_in[t % NBX], 16 * (t // NBX + 1))
                if t >= 1:
                    scalar.wait_ge(sem_v, t)  # D'(t-1) retired: Oh free
                xv = X[t % NBX][:].rearrange("p (k t m) -> p k t m", t=2, m=512)
                Ov = xv[:, :, 1, :]
                Ohv = OH[:].rearrange("p (k m) -> p k m", m=512)
                nc.scalar.mul(Ohv, Ov, 0.5).then_inc(sem_act, 1)
                if t >= 1:
                    out_dma(scalar, t - 1)
            out_dma(scalar, n_tiles - 1)
            # drain: don't let the kernel end with output DMAs in flight
            for i in range(NBO):
                n_dmas = len(range(i, n_tiles, NBO))
                if n_dmas:
                    scalar.wait_ge(sem_out[i], 16 * n_dmas)

        @block.vector
        def _(vector):
            for t in range(n_tiles):
                Xt = X[t % NBX]
                Ot = OUT[t % NBO]

                vector.wait_ge(sem_in[t % NBX], 16 * (t // NBX + 1))
                vector.wait_ge(sem_act, t + 1)  # Oh(t) ready (ACT retired)
                if t >= 1:
                    # stage 2 of t-1 retired: its reads of S are done, so
                    # S'(t) may overwrite it (same-engine WAR also needs an
                    # explicit retire wait)
                    vector.wait_ge(sem_dve, t)
                xv = Xt[:].rearrange("p (k t m) -> p k t m", t=2, m=512)
                E = xv[:, :, 0, :]
                Dv = xv[:, :, 1, :]  # D overwrites X's odd rows
                Ohv = OH[:].rearrange("p (k m) -> p k m", m=512)
                Sv = S[:].rearrange("p (k m) -> p k m", m=512)
                # S = 0.5*E + Oh, D = 0.5*E - Oh (Oh = 0.5*O from ACT)
                nc.vector.scalar_tensor_tensor(
                    Sv, E, 0.5, Ohv, mybir.AluOpType.mult, mybir.AluOpType.add
                )
                nc.vector.scalar_tensor_tensor(
                    Dv, E, 0.5, Ohv, mybir.AluOpType.mult, mybir.AluOpType.subtract
                ).then_inc(sem_v, 1)
                vector.wait_ge(sem_v, t + 1)  # S'/D' retired (in order)

                if t >= NBO:
                    # OUT[t % NBO] was drained by the out-DMA of t - NBO
                    vector.wait_ge(sem_out[t % NBO], 16 * (t // NBO))
                Sp = S[:].rearrange("p (k m t) -> p k t m", t=2, m=256)
                Xr = Xt[:].rearrange(
                    "p (k t m w) -> p k t w m", t=2, m=256, w=2
                )
                Se, So = Sp[:, :, 0, :], Sp[:, :, 1, :]
                De, Do = Xr[:, :, 1, 0, :], Xr[:, :, 1, 1, :]
                # quadrant order matches OUT_NAMES / y's q axis
                ov = Ot[:].rearrange("p (q k m) -> p q k m", q=4, m=256)
                nc.vector.tensor_add(ov[:, 0], Se, So)  # LL
                nc.vector.tensor_add(ov[:, 1], De, Do)  # LH
                nc.vector.tensor_sub(ov[:, 2], Se, So)  # HL
                nc.vector.tensor_sub(ov[:, 3], De, Do).then_inc(sem_dve, 1)  # HH

    return nc


def _shard_inputs(x: np.ndarray) -> list[dict[str, np.ndarray]]:
    x = np.ascontiguousarray(np.asarray(x, dtype=np.float32))
    assert x.shape == (B, C, H, W), x.shape
    return [
        {
            "x": x[i * PER_CORE_B : (i + 1) * PER_CORE_B].reshape(
                N_TILES, 128, TILE_FREE
            )
        }
        for i in range(N_CORES)
    ]


def _gather(results: list[dict[str, np.ndarray]]) -> dict[str, np.ndarray]:
    full = {}
    for q, name in enumerate(OUT_NAMES):
        full[name] = np.concatenate(
            [
                results[i]["y"][q].reshape(PER_CORE_B, C, H // 2, W // 2)
                for i in range(N_CORES)
            ],
            axis=0,
        )
    return full


def _run(x: np.ndarray, **spmd_kwargs):
    nc = _build_kernel()
    in_maps = _shard_inputs(x)
    out = run_bass_kernel_spmd(nc, in_maps, list(range(N_CORES)), **spmd_kwargs)
    return _gather(out.results), out


def kernel(x: np.ndarray):
    full, _ = _run(x)
    return (full["ll"], (full["lh"], full["hl"], full["hh"]))
